# revision 1
# baseline (speedup 1.0000x reference)
"""Trainium2 Bass kernel for nn_DecoderBlock_17265768530695.

8 NeuronCores: data-parallel over batch (2) x tensor-parallel over heads (4).
Core c handles batch b=c//4, head t=c%4. TP groups: [[0,1,2,3],[4,5,6,7]].

Algebra (all biases are zero by the input spec):
  xr     = rope(x)
  logits = xr @ G @ xr^T        G = ws(wq)@ws(mwq_t)@ws(mwk_t)^T@ws(wk)^T/32
  ctx    = softmax(causal(logits)) @ (xr @ Vt)     Vt = ws(wv)@ws(mwv_t)
  out    = diag(s) @ ctx @ Wof_t   summed over heads via ReduceScatter,
           Wof_t = ws(mwo)[rows_t,:] @ ws(wf),  s = rowsum(x)*rowsum(xr)/4

Precision: q/k/logits path uses f32r hi/lo splits (3 matmuls per product,
fp32-class accuracy at full PE rate); v/output paths single f32r / fp16.
"""

import contextlib

import numpy as np

import concourse.bacc as bacc
import concourse.tile as tile
import concourse.mybir as mybir
from concourse.bass_utils import run_bass_kernel_spmd
from concourse.masks import make_identity

f32 = mybir.dt.float32
f32r = mybir.dt.float32r
f16 = mybir.dt.float16
FP = mybir.ActivationFunctionType
X = mybir.AxisListType.X

P = 128
D = 1024
NSEQ = 2048
ZT = 1024
ZF = 4096
DC = D // P       # 8
NCH = NSEQ // P   # 16
CC = ZF // P      # 32
EPS = 1e-5
HB = 512          # fold half width (pair-split)
GROUPS = [[0, 1, 2, 3], [4, 5, 6, 7]]
PAIRS = [[0, 4], [1, 5], [2, 6], [3, 7]]
ALL8 = [[0, 1, 2, 3, 4, 5, 6, 7]]
N_CORES = 8

_CACHE = {}


def _split(nc, hi, lo, src_fp32):
    """hi = f32r(src); lo = f32r(src - hi). src may be PSUM or SBUF fp32."""
    nc.scalar.copy(hi, src_fp32)
    nc.vector.tensor_sub(lo, src_fp32, hi.bitcast(f32))


def _colstats(nc, pool, w, isg_out, nb_out, n_free):
    """Per-partition stats over the free dim of fp32 tile w [128, n_free].
    isg_out <- 1/sqrt(var+eps); nb_out <- -mu/sqrt(var+eps)."""
    msum = pool.tile([P, 1], f32, tag="st_ms")
    nc.vector.reduce_sum(msum[:], w, axis=X)
    nc.vector.tensor_scalar_mul(msum[:], msum[:], 1.0 / n_free)
    sq = pool.tile([P, n_free], f32, tag="st_sq")
    nc.vector.tensor_mul(sq[:], w, w)
    vs = pool.tile([P, 1], f32, tag="st_vs")
    nc.vector.reduce_sum(vs[:], sq[:], axis=X)
    nc.vector.tensor_scalar_mul(vs[:], vs[:], 1.0 / n_free)
    msq = pool.tile([P, 1], f32, tag="st_msq")
    nc.vector.tensor_mul(msq[:], msum[:], msum[:])
    nc.vector.tensor_sub(vs[:], vs[:], msq[:])
    nc.vector.tensor_scalar_add(vs[:], vs[:], EPS)
    nc.scalar.activation(vs[:], vs[:], FP.Sqrt)
    nc.vector.reciprocal(isg_out, vs[:])
    nc.vector.tensor_mul(msq[:], msum[:], isg_out)
    nc.vector.tensor_scalar_mul(nb_out, msq[:], -1.0)


def _rowstats(nc, pool, name, psum_s, psum_q, n_rows, width):
    """Finish [1,width] column stats from accumulated sum/sumsq PSUMs.
    Returns (mu_f32, invsig_f32) SBUF [1,width] tiles."""
    mu = pool.tile([1, width], f32, tag=f"{name}_mu")
    t1 = pool.tile([1, width], f32, tag=f"{name}_t1")
    for q in range(width // 512):
        sl = slice(q * 512, (q + 1) * 512)
        nc.scalar.copy(mu[:, sl], psum_s[q][:])
        nc.scalar.copy(t1[:, sl], psum_q[q][:])
    nc.vector.tensor_scalar_mul(mu[:], mu[:], 1.0 / n_rows)
    nc.vector.tensor_scalar_mul(t1[:], t1[:], 1.0 / n_rows)
    msq = pool.tile([1, width], f32, tag=f"{name}_msq")
    nc.vector.tensor_mul(msq[:], mu[:], mu[:])
    nc.vector.tensor_sub(t1[:], t1[:], msq[:])
    nc.vector.tensor_scalar_add(t1[:], t1[:], EPS)
    nc.scalar.activation(t1[:], t1[:], FP.Sqrt)
    isg = pool.tile([1, width], f32, tag=f"{name}_isg")
    nc.vector.reciprocal(isg[:], t1[:])
    return mu, isg


def build_nc():
    nc = bacc.Bacc("TRN2", target_bir_lowering=False, debug=False,
                   num_devices=N_CORES)

    xT = nc.declare_dram_parameter("xT", [D, NSEQ], f32, isOutput=False)
    cosT = nc.declare_dram_parameter("cosT", [D // 2, NSEQ], f32, isOutput=False)
    sinT = nc.declare_dram_parameter("sinT", [D // 2, NSEQ], f32, isOutput=False)
    wqT = nc.declare_dram_parameter("wqT", [D, D], f32, isOutput=False)
    wkT = nc.declare_dram_parameter("wkT", [D, D], f32, isOutput=False)
    wvT = nc.declare_dram_parameter("wvT", [D, D], f32, isOutput=False)
    mwqT = nc.declare_dram_parameter("mwqT", [ZT, D], f32, isOutput=False)
    mwk = nc.declare_dram_parameter("mwk", [D, ZT], f32, isOutput=False)
    mwv = nc.declare_dram_parameter("mwv", [D, ZT], f32, isOutput=False)
    mwoT = nc.declare_dram_parameter("mwoT", [ZF, HB], f32, isOutput=False)
    wf = nc.declare_dram_parameter("wf", [ZF, D], f32, isOutput=False)
    maskc = nc.declare_dram_parameter("maskc", [P, P], f32, isOutput=False)
    out_shards = [nc.declare_dram_parameter(f"out_shard{g}", [P, D], f32,
                                            isOutput=True) for g in range(4)]

    with tile.TileContext(nc) as tc:
        with contextlib.ExitStack() as est:
            dram = est.enter_context(
                tc.tile_pool(name="dram", bufs=1, space="DRAM"))
            persist = est.enter_context(tc.tile_pool(name="persist", bufs=1))

            xrhi_d = dram.tile([D, NSEQ], f32r, tag="xrhi_d")
            xrlo_d = dram.tile([D, NSEQ], f32r, tag="xrlo_d")
            uhi_d = dram.tile([D, NSEQ], f32r, tag="uhi_d")
            ulo_d = dram.tile([D, NSEQ], f32r, tag="ulo_d")
            kthi_d = dram.tile([ZT, HB], f32r, tag="kthi_d")
            ktlo_d = dram.tile([ZT, HB], f32r, tag="ktlo_d")
            t2hi_d = dram.tile([D, HB], f32r, tag="t2hi_d")
            t2lo_d = dram.tile([D, HB], f32r, tag="t2lo_d")
            s_d = dram.tile([1, NSEQ], f32, tag="s_d")
            mqshi_d = dram.tile([ZT, D], f32r, tag="mqshi_d")
            mqslo_d = dram.tile([ZT, D], f32r, tag="mqslo_d")
            wqshi_d = dram.tile([D, D], f32r, tag="wqshi_d")
            wqslo_d = dram.tile([D, D], f32r, tag="wqslo_d")
            mkhi_d = dram.tile([D, ZT], f32r, tag="mkhi_d")
            mklo_d = dram.tile([D, ZT], f32r, tag="mklo_d")
            kisg_d = dram.tile([1, ZT], f32, tag="kisg_d")
            visg_d = dram.tile([1, ZT], f32, tag="visg_d")
            ar_in_d = dram.tile([2, ZF], f32, tag="ar_in_d")
            ar_out_d = dram.tile([2, ZF], f32, tag="ar_out_d")
            vtx_in = dram.tile([HB, ZT], f32, tag="vtx_in")
            vtx_out = dram.tile([D, ZT], f32, tag="vtx_out")
            wfx_in = dram.tile([HB, D], f32, tag="wfx_in")
            wfx_out = dram.tile([D, D], f32, tag="wfx_out")
            gx_in = dram.tile([D, HB], f32, tag="gx_in")
            gx_out = dram.tile([2 * D, HB], f32, tag="gx_out")
            vh_d = dram.tile([NSEQ, ZT], f16, tag="vh_d")
            outg_d = [dram.tile([512, D], f32, tag=f"outg{g}",
                                name=f"outg{g}") for g in range(4)]
            rsg_d = [dram.tile([P, D], f32, tag=f"rsg{g}",
                               name=f"rsg{g}") for g in range(4)]

            ident32 = persist.tile([P, P], f32, tag="ident32")
            make_identity(nc, ident32[:])
            ident_r = persist.tile([P, P], f32r, tag="ident_r")
            nc.vector.tensor_copy(ident_r[:], ident32[:])
            ident16 = persist.tile([P, P], f16, tag="ident16")
            nc.vector.tensor_copy(ident16[:], ident32[:])
            mask_sb = persist.tile([P, P], f32, tag="mask_sb")
            nc.sync.dma_start(mask_sb[:], maskc[:])
            ones_col = persist.tile([P, 1], f32, tag="ones_col")
            nc.any.memset(ones_col[:], 1.0)
            ones_col_r = persist.tile([P, 1], f32r, tag="ones_col_r")
            nc.vector.tensor_copy(ones_col_r[:], ones_col[:])
            ones_row = persist.tile([1, P], f32, tag="ones_row")
            nc.any.memset(ones_row[:], 1.0)
            ones_row_r = persist.tile([1, P], f32r, tag="ones_row_r")
            nc.vector.tensor_copy(ones_row_r[:], ones_row[:])

            s_sb = persist.tile([P, NCH], f32, tag="s_sb")
            coef_sb = persist.tile([P, NCH], f32, tag="coef_sb")
            o_scale = persist.tile([P, CC], f32, tag="o_scale")
            o_bias = persist.tile([P, CC], f32, tag="o_bias")

            with tc.tile_pool(name="vtk", bufs=1) as vtk:
                vt_sb = vtk.tile([P, DC, ZT], f32r, tag="vt_sb")

                # ======== P0: mwo column stats (my z-half) + AllReduce =======
                # mwoT input is the z-half slice; the 8-core AllReduce sums
                # over all 4 heads x both halves.
                with tc.tile_pool(name="p0", bufs=2) as p0, \
                     tc.tile_pool(name="p0a", bufs=1) as p0a:
                    ssum = p0a.tile([P, CC], f32, tag="ssum")
                    ssq = p0a.tile([P, CC], f32, tag="ssq")
                    for cc in range(CC):
                        mo = p0.tile([P, HB], f32, tag="mo")
                        nc.sync.dma_start(mo[:], mwoT[cc * P:(cc + 1) * P, :])
                        nc.vector.reduce_sum(ssum[:, cc:cc + 1], mo[:], axis=X)
                        sqd = p0.tile([P, HB], f32, tag="mosq")
                        nc.scalar.activation(sqd[:], mo[:], FP.Square,
                                             accum_out=ssq[:, cc:cc + 1])
                    nc.sync.dma_start(
                        ar_in_d[0:1, :].rearrange("o (c p) -> (o p) c", p=P),
                        ssum[:])
                    nc.sync.dma_start(
                        ar_in_d[1:2, :].rearrange("o (c p) -> (o p) c", p=P),
                        ssq[:])
                    nc.gpsimd.collective_compute(
                        "AllReduce", mybir.AluOpType.add,
                        ins=[ar_in_d.opt()], outs=[ar_out_d.opt()],
                        replica_groups=ALL8)

                # ---- Vt fold (single precision) ----
                with tc.tile_pool(name="vf", bufs=2) as vf, \
                     tc.tile_pool(name="vfs", bufs=1) as vfs, \
                     tc.tile_pool(name="vpa", bufs=1, space="PSUM") as vpa, \
                     tc.tile_pool(name="vpw", bufs=2, space="PSUM") as vpw:
                    wvs_r = vfs.tile([P, DC, HB], f32r, tag="wvs_r")
                    for cc in range(DC):
                        w = vf.tile([P, D], f32, tag="wvT_c")
                        nc.sync.dma_start(w[:], wvT[cc * P:(cc + 1) * P, :])
                        isg = vf.tile([P, 1], f32, tag="wv_isg")
                        nb = vf.tile([P, 1], f32, tag="wv_nb")
                        _colstats(nc, vf, w[:], isg[:], nb[:], D)
                        nc.vector.tensor_scalar(
                            out=wvs_r[:, cc], in0=w[:, 0:HB], scalar1=isg[:],
                            scalar2=nb[:], op0=mybir.AluOpType.mult,
                            op1=mybir.AluOpType.add)
                    mwv_r = vfs.tile([P, DC, ZT], f32r, tag="mwv_r")
                    ps_vs = [vpa.tile([1, 512], f32, tag=f"vs{q}",
                                      name=f"vs{q}") for q in range(2)]
                    ps_vq = [vpa.tile([1, 512], f32, tag=f"vq{q}",
                                      name=f"vq{q}") for q in range(2)]
                    for cc in range(DC):
                        w = vf.tile([P, ZT], f32, tag="mwv_c")
                        nc.sync.dma_start(w[:], mwv[cc * P:(cc + 1) * P, :])
                        nc.scalar.copy(mwv_r[:, cc], w[:])
                        sq = vf.tile([P, ZT], f32r, tag="mwv_sq")
                        nc.vector.tensor_mul(sq[:], w[:], w[:])
                        first, last = cc == 0, cc == DC - 1
                        for q in range(2):
                            sl = slice(q * 512, (q + 1) * 512)
                            nc.tensor.matmul(ps_vs[q][:], ones_col_r[:],
                                             mwv_r[:, cc, sl],
                                             start=first, stop=last)
                            nc.tensor.matmul(ps_vq[q][:], ones_col_r[:],
                                             sq[:, sl], start=first,
                                             stop=last)
                    v_mu, v_isg = _rowstats(nc, vfs, "v", ps_vs, ps_vq, D, ZT)
                    v_mu_r = vfs.tile([1, ZT], f32r, tag="v_mu_r")
                    nc.vector.tensor_copy(v_mu_r[:], v_mu[:])
                    v_isg_r = vfs.tile([1, ZT], f32r, tag="v_isg_r")
                    nc.vector.tensor_copy(v_isg_r[:], v_isg[:])
                    mub_v = vfs.tile([P, ZT], f32, tag="mub_v")
                    isgb_v = vfs.tile([P, ZT], f32, tag="isgb_v")
                    for q in range(2):
                        sl = slice(q * 512, (q + 1) * 512)
                        pmv = vpw.tile([P, 512], f32, tag="pmv", bufs=1)
                        nc.tensor.matmul(pmv[:], ones_row_r[:], v_mu_r[:, sl],
                                         start=True, stop=True)
                        nc.scalar.copy(mub_v[:, sl], pmv[:])
                        piv = vpw.tile([P, 512], f32, tag="piv", bufs=1)
                        nc.tensor.matmul(piv[:], ones_row_r[:],
                                         v_isg_r[:, sl], start=True,
                                         stop=True)
                        nc.scalar.copy(isgb_v[:, sl], piv[:])
                    for cc in range(DC):
                        nc.vector.tensor_sub(mwv_r[:, cc],
                                             mwv_r[:, cc].bitcast(f32),
                                             mub_v[:])
                    # local Vt rows 0:HB == true rows b*HB:(b+1)*HB (wvT's
                    # fan-in halves are swapped on cores 4-7); visg is folded
                    # into Vt columns here (not into Wof rows).
                    for mc in range(DC // 2):
                        for st in range(2):
                            ssl = slice(st * 512, (st + 1) * 512)
                            pv = vpw.tile([P, 512], f32, tag="pv")
                            for cc in range(DC):
                                nc.tensor.matmul(
                                    pv[:], wvs_r[:, cc, mc * P:(mc + 1) * P],
                                    mwv_r[:, cc, ssl],
                                    start=(cc == 0), stop=(cc == DC - 1))
                            vtx_e = vf.tile([P, 512], f32, tag="vtx_e")
                            nc.vector.tensor_mul(vtx_e[:], pv[:],
                                                 isgb_v[:, ssl])
                            nc.sync.dma_start(
                                vtx_in[mc * P:(mc + 1) * P, ssl], vtx_e[:])
                    nc.gpsimd.collective_compute(
                        "AllGather", mybir.AluOpType.bypass,
                        ins=[vtx_in.opt()], outs=[vtx_out.opt()],
                        replica_groups=PAIRS)

                # ======== P0 finish: o_scale/o_bias from AllReduce ===========
                with tc.tile_pool(name="p0s", bufs=1) as p0s:
                    gsum = p0s.tile([P, CC], f32, tag="gsum")
                    gsq = p0s.tile([P, CC], f32, tag="gsq")
                    nc.sync.dma_start(
                        gsum[:],
                        ar_out_d[0:1, :].rearrange("o (c p) -> (o p) c", p=P))
                    nc.sync.dma_start(
                        gsq[:],
                        ar_out_d[1:2, :].rearrange("o (c p) -> (o p) c", p=P))
                    mu = p0s.tile([P, CC], f32, tag="mu_o")
                    nc.vector.tensor_scalar_mul(mu[:], gsum[:], 1.0 / ZF)
                    var = p0s.tile([P, CC], f32, tag="var_o")
                    nc.vector.tensor_scalar_mul(var[:], gsq[:], 1.0 / ZF)
                    musq = p0s.tile([P, CC], f32, tag="musq_o")
                    nc.vector.tensor_mul(musq[:], mu[:], mu[:])
                    nc.vector.tensor_sub(var[:], var[:], musq[:])
                    nc.vector.tensor_scalar_add(var[:], var[:], EPS)
                    nc.scalar.activation(var[:], var[:], FP.Sqrt)
                    nc.vector.reciprocal(o_scale[:], var[:])
                    nc.vector.tensor_mul(o_bias[:], mu[:], o_scale[:])
                    nc.vector.tensor_scalar_mul(o_bias[:], o_bias[:], -1.0)

                # ======== Wof fold early (spilled to DRAM) ===============
                with tc.tile_pool(name="w6s", bufs=1) as w6s:
                    with tc.tile_pool(name="w6", bufs=2) as w6, \
                         tc.tile_pool(name="w6ps", bufs=1,
                                      space="PSUM") as w6p:
                        ps_fs = [w6p.tile([1, 512], f32, tag=f"fs{q}",
                                          name=f"fs{q}") for q in range(2)]
                        ps_fq = [w6p.tile([1, 512], f32, tag=f"fq{q}",
                                          name=f"fq{q}") for q in range(2)]
                        for cc in range(CC):
                            w = w6.tile([P, D], f32, tag="wf_c")
                            nc.sync.dma_start(w[:],
                                              wf[cc * P:(cc + 1) * P, :])
                            wr = w6.tile([P, D], f32r, tag="wf_r")
                            nc.scalar.copy(wr[:], w[:])
                            sq = w6.tile([P, D], f32r, tag="wf_sq")
                            nc.vector.tensor_mul(sq[:], w[:], w[:])
                            first, last = cc == 0, cc == CC - 1
                            for q in range(2):
                                sl = slice(q * 512, (q + 1) * 512)
                                nc.tensor.matmul(ps_fs[q][:], ones_col_r[:],
                                                 wr[:, sl], start=first,
                                                 stop=last)
                                nc.tensor.matmul(ps_fq[q][:], ones_col_r[:],
                                                 sq[:, sl], start=first,
                                                 stop=last)
                        f_mu, f_isg = _rowstats(nc, w6s, "f", ps_fs, ps_fq,
                                                ZF, D)
                        f_mu_r = w6s.tile([1, D], f32r, tag="f_mu_r")
                        nc.vector.tensor_copy(f_mu_r[:], f_mu[:])
                        f_isg_r = w6s.tile([1, D], f32r, tag="f_isg_r")
                        nc.vector.tensor_copy(f_isg_r[:], f_isg[:])
                        mub_f = w6s.tile([P, D], f32, tag="mub_f")
                        isgb_f = w6s.tile([P, D], f32, tag="isgb_f")
                        for q in range(2):
                            sl = slice(q * 512, (q + 1) * 512)
                            pm = w6p.tile([P, 512], f32, tag="pm_f",
                                          name=f"pm_f{q}")
                            nc.tensor.matmul(pm[:], ones_row_r[:],
                                             f_mu_r[:, sl], start=True,
                                             stop=True)
                            nc.scalar.copy(mub_f[:, sl], pm[:])
                            pi = w6p.tile([P, 512], f32, tag="pi_f",
                                          name=f"pi_f{q}")
                            nc.tensor.matmul(pi[:], ones_row_r[:],
                                             f_isg_r[:, sl], start=True,
                                             stop=True)
                            nc.scalar.copy(isgb_f[:, sl], pi[:])
                    for jp in range(2):
                        jsl = slice(jp * 512, (jp + 1) * 512)
                        with tc.tile_pool(name=f"w6h{jp}", bufs=1) as w6h, \
                             tc.tile_pool(name=f"w6f{jp}", bufs=2) as w6f, \
                             tc.tile_pool(name=f"w6fp{jp}", bufs=1,
                                          space="PSUM") as w6fp:
                            wfh = w6h.tile([P, CC, 512], f32r, tag="wfh")
                            for cc in range(CC):
                                w = w6f.tile([P, 512], f32, tag="wfh_c")
                                nc.sync.dma_start(
                                    w[:], wf[cc * P:(cc + 1) * P, jsl])
                                nc.vector.tensor_sub(w[:], w[:],
                                                     mub_f[:, jsl])
                                nc.vector.tensor_mul(wfh[:, cc], w[:],
                                                     isgb_f[:, jsl])
                            pws = [w6fp.tile([P, 512], f32, tag=f"pw{ic}",
                                             name=f"pw{ic}")
                                   for ic in range(DC // 2)]
                            for cc in range(CC):
                                mo = w6f.tile([P, HB], f32, tag="mo_f")
                                nc.sync.dma_start(
                                    mo[:], mwoT[cc * P:(cc + 1) * P, :])
                                mos = w6f.tile([P, HB], f32r, tag="mos_f")
                                nc.vector.tensor_scalar(
                                    out=mos[:], in0=mo[:],
                                    scalar1=o_scale[:, cc:cc + 1],
                                    scalar2=o_bias[:, cc:cc + 1],
                                    op0=mybir.AluOpType.mult,
                                    op1=mybir.AluOpType.add)
                                for ic in range(DC // 2):
                                    nc.tensor.matmul(
                                        pws[ic][:],
                                        mos[:, ic * P:(ic + 1) * P],
                                        wfh[:, cc],
                                        start=(cc == 0), stop=(cc == CC - 1))
                            for ic in range(DC // 2):
                                we_ = w6f.tile([P, 512], f32, tag="wof_e")
                                nc.scalar.copy(we_[:], pws[ic][:])
                                nc.sync.dma_start(
                                    wfx_in[ic * P:(ic + 1) * P, jsl], we_[:])
                    nc.gpsimd.collective_compute(
                        "AllGather", mybir.AluOpType.bypass,
                        ins=[wfx_in.opt()], outs=[wfx_out.opt()],
                        replica_groups=PAIRS)

                # ======== P2b: mwk stats + centered pre-split to DRAM =========
                with tc.tile_pool(name="km", bufs=1) as km:
                    k_isg_pp = km.tile([P, DC], f32, tag="k_isg_pp")
                    with tc.tile_pool(name="mkst", bufs=2) as fs, \
                         tc.tile_pool(name="mkss", bufs=1) as fss, \
                         tc.tile_pool(name="mkps", bufs=1, space="PSUM") as fps:
                        ps_s = [fps.tile([1, 512], f32, tag=f"ks{q}",
                                         name=f"ks{q}") for q in range(2)]
                        ps_q = [fps.tile([1, 512], f32, tag=f"kq{q}",
                                         name=f"kq{q}") for q in range(2)]
                        for cc in range(DC):
                            mk = fs.tile([P, ZT], f32, tag="mk")
                            nc.sync.dma_start(mk[:], mwk[cc * P:(cc + 1) * P, :])
                            hi = fs.tile([P, ZT], f32r, tag="mk_hi")
                            lo = fs.tile([P, ZT], f32r, tag="mk_lo")
                            _split(nc, hi[:], lo[:], mk[:])
                            sq = fs.tile([P, ZT], f32, tag="mk_sq")
                            nc.vector.tensor_mul(sq[:], mk[:], mk[:])
                            shi = fs.tile([P, ZT], f32r, tag="mk_shi")
                            slo = fs.tile([P, ZT], f32r, tag="mk_slo")
                            _split(nc, shi[:], slo[:], sq[:])
                            first, last = cc == 0, cc == DC - 1
                            for q in range(2):
                                sl = slice(q * 512, (q + 1) * 512)
                                nc.tensor.matmul(ps_s[q][:], ones_col_r[:],
                                                 hi[:, sl], start=first,
                                                 stop=False)
                                nc.tensor.matmul(ps_s[q][:], ones_col_r[:],
                                                 lo[:, sl], start=False,
                                                 stop=last)
                                nc.tensor.matmul(ps_q[q][:], ones_col_r[:],
                                                 shi[:, sl], start=first,
                                                 stop=False)
                                nc.tensor.matmul(ps_q[q][:], ones_col_r[:],
                                                 slo[:, sl], start=False,
                                                 stop=last)
                        k_mu, k_isg = _rowstats(nc, fss, "k", ps_s, ps_q, D, ZT)
                        k_mu_hi = fss.tile([1, ZT], f32r, tag="k_mu_hi")
                        k_mu_lo = fss.tile([1, ZT], f32r, tag="k_mu_lo")
                        _split(nc, k_mu_hi[:], k_mu_lo[:], k_mu[:])
                        nc.sync.dma_start(kisg_d[:], k_isg[:])
                        nc.sync.dma_start(
                            k_isg_pp[:],
                            kisg_d.opt().rearrange("o (a p) -> (o p) a", p=P))
                        mub_k = fss.tile([P, ZT], f32, tag="mub_k")
                        for q in range(2):
                            sl = slice(q * 512, (q + 1) * 512)
                            pmu = fps.tile([P, 512], f32, tag="pmu_k",
                                           name=f"pmu_k{q}")
                            nc.tensor.matmul(pmu[:], ones_row_r[:],
                                             k_mu_hi[:, sl], start=True,
                                             stop=False)
                            nc.tensor.matmul(pmu[:], ones_row_r[:],
                                             k_mu_lo[:, sl], start=False,
                                             stop=True)
                            nc.scalar.copy(mub_k[:, sl], pmu[:])
                        for cc in range(DC):
                            mk = fs.tile([P, ZT], f32, tag="mk")
                            nc.sync.dma_start(mk[:], mwk[cc * P:(cc + 1) * P, :])
                            cen = fs.tile([P, ZT], f32, tag="mk_cen")
                            nc.vector.tensor_sub(cen[:], mk[:], mub_k[:])
                            chi = fs.tile([P, ZT], f32r, tag="mk_chi")
                            clo = fs.tile([P, ZT], f32r, tag="mk_clo")
                            _split(nc, chi[:], clo[:], cen[:])
                            nc.sync.dma_start(mkhi_d[cc * P:(cc + 1) * P, :],
                                              chi[:])
                            nc.sync.dma_start(mklo_d[cc * P:(cc + 1) * P, :],
                                              clo[:])

                    # ======== P2c: wksT resident + KtT fold ===================
                    with tc.tile_pool(name="wksp", bufs=1) as wksp, \
                         tc.tile_pool(name="wkw", bufs=2) as wkw, \
                         tc.tile_pool(name="wkb", bufs=3) as wkb, \
                         tc.tile_pool(name="ktps", bufs=3, space="PSUM") as kp:
                        wks_hi = wksp.tile([P, DC, HB], f32r, tag="wks_hi")
                        wks_lo = wksp.tile([P, DC, HB], f32r, tag="wks_lo")
                        for cc in range(DC):
                            w = wkw.tile([P, D], f32, tag="wkT_c")
                            nc.sync.dma_start(w[:], wkT[cc * P:(cc + 1) * P, :])
                            isg = wkw.tile([P, 1], f32, tag="wk_isg")
                            nb = wkw.tile([P, 1], f32, tag="wk_nb")
                            _colstats(nc, wkw, w[:], isg[:], nb[:], D)
                            std = wkw.tile([P, HB], f32, tag="wk_std")
                            nc.vector.tensor_scalar(
                                out=std[:], in0=w[:, 0:HB], scalar1=isg[:],
                                scalar2=nb[:], op0=mybir.AluOpType.mult,
                                op1=mybir.AluOpType.add)
                            _split(nc, wks_hi[:, cc], wks_lo[:, cc], std[:])
                        # ======== P2a: pre-split mwqT / wqT to DRAM ==================
                        for W_IN, HI_D, LO_D in ((mwqT, mqshi_d, mqslo_d),
                                                 (wqT, wqshi_d, wqslo_d)):
                            with tc.tile_pool(name="pre", bufs=1) as pre:
                                for ccp in range(DC):
                                    w = pre.tile([P, D], f32, tag="pre_w")
                                    nc.sync.dma_start(w[:], W_IN[ccp * P:(ccp + 1) * P, :])
                                    isg = pre.tile([P, 1], f32, tag="pre_isg")
                                    nb = pre.tile([P, 1], f32, tag="pre_nb")
                                    _colstats(nc, pre, w[:], isg[:], nb[:], D)
                                    std = pre.tile([P, D], f32, tag="pre_std")
                                    nc.vector.tensor_scalar(
                                        out=std[:], in0=w[:], scalar1=isg[:],
                                        scalar2=nb[:], op0=mybir.AluOpType.mult,
                                        op1=mybir.AluOpType.add)
                                    hi = pre.tile([P, D], f32r, tag="pre_hi")
                                    lo = pre.tile([P, D], f32r, tag="pre_lo")
                                    _split(nc, hi[:], lo[:], std[:])
                                    nc.sync.dma_start(HI_D[ccp * P:(ccp + 1) * P, :],
                                                      hi[:])
                                    nc.sync.dma_start(LO_D[ccp * P:(ccp + 1) * P, :],
                                                      lo[:])

                        for jc in range(DC):
                            jsl = slice(jc * P, (jc + 1) * P)
                            pk = kp.tile([P, 512], f32, tag="pk")
                            for cc in range(DC):
                                bh_ = wkb.tile([P, P], f32r, tag="mkb_hi")
                                bl_ = wkb.tile([P, P], f32r, tag="mkb_lo")
                                nc.sync.dma_start(
                                    bh_[:], mkhi_d[cc * P:(cc + 1) * P, jsl])
                                nc.sync.dma_start(
                                    bl_[:], mklo_d[cc * P:(cc + 1) * P, jsl])
                                nc.tensor.matmul(pk[:], bh_[:],
                                                 wks_hi[:, cc],
                                                 start=(cc == 0), stop=False)
                                nc.tensor.matmul(pk[:], bh_[:],
                                                 wks_lo[:, cc],
                                                 start=False, stop=False)
                                nc.tensor.matmul(pk[:], bl_[:],
                                                 wks_hi[:, cc],
                                                 start=False,
                                                 stop=(cc == DC - 1))
                            std32 = wkw.tile([P, 512], f32, tag="kt32")
                            nc.scalar.activation(
                                std32[:], pk[:], FP.Copy,
                                scale=k_isg_pp[:, jc:jc + 1])
                            hi_e = wkw.tile([P, 512], f32r, tag="kthi_e")
                            lo_e = wkw.tile([P, 512], f32r, tag="ktlo_e")
                            _split(nc, hi_e[:], lo_e[:], std32[:])
                            nc.sync.dma_start(
                                kthi_d[jc * P:(jc + 1) * P, :], hi_e[:])
                            nc.sync.dma_start(
                                ktlo_d[jc * P:(jc + 1) * P, :], lo_e[:])

                # ======== rope (P1) nested with T2 fold for overlap ===========
                # rowsums of x and rope(x) fused into the rope loop (PSUM
                # column-sum via ones^T matmuls) — saves a 16MB reload later.
                with tc.tile_pool(name="p1io", bufs=2) as p1io, \
                     tc.tile_pool(name="p1c", bufs=1) as p1c, \
                     tc.tile_pool(name="t2s", bufs=1) as t2s, \
                     tc.tile_pool(name="t2b", bufs=3) as t2b, \
                     tc.tile_pool(name="t2w", bufs=2) as t2w, \
                     tc.tile_pool(name="t2ps", bufs=4, space="PSUM") as tp:
                    for dc in range(DC // 2):
                        x1 = p1io.tile([P, NSEQ], f32, tag="x1")
                        x2 = p1io.tile([P, NSEQ], f32, tag="x2")
                        nc.gpsimd.dma_start(x1[:], xT[dc * P:(dc + 1) * P, :])
                        nc.gpsimd.dma_start(x2[:], xT[(dc + 4) * P:(dc + 5) * P, :])
                        co = p1io.tile([P, NSEQ], f32, tag="co", bufs=1)
                        si = p1io.tile([P, NSEQ], f32, tag="si", bufs=1)
                        nc.gpsimd.dma_start(co[:], cosT[dc * P:(dc + 1) * P, :])
                        nc.gpsimd.dma_start(si[:], sinT[dc * P:(dc + 1) * P, :])
                        xr1 = p1c.tile([P, NSEQ], f32, tag="xr1")
                        xr2 = p1c.tile([P, NSEQ], f32, tag="xr2")
                        m2 = p1c.tile([P, NSEQ], f32, tag="m2")
                        nc.vector.tensor_mul(xr1[:], x1[:], co[:])
                        nc.vector.tensor_mul(m2[:], x2[:], si[:])
                        nc.vector.tensor_sub(xr1[:], xr1[:], m2[:])
                        nc.vector.tensor_mul(xr2[:], x2[:], co[:])
                        nc.vector.tensor_mul(m2[:], x1[:], si[:])
                        nc.vector.tensor_add(xr2[:], xr2[:], m2[:])
                        for half, xr_ in ((0, xr1), (1, xr2)):
                            hi = p1c.tile([P, NSEQ], f32r, tag="xrhi")
                            lo = p1c.tile([P, NSEQ], f32r, tag="xrlo")
                            _split(nc, hi[:], lo[:], xr_[:])
                            row = dc + 4 * half
                            nc.gpsimd.dma_start(xrhi_d[row * P:(row + 1) * P, :],
                                              hi[:])
                            nc.gpsimd.dma_start(xrlo_d[row * P:(row + 1) * P, :],
                                              lo[:])
                    # ---- T2 fold: lhsT blocks pre-split in DRAM ----
                    kth = t2s.tile([P, DC, 512], f32r, tag="kth")
                    ktl = t2s.tile([P, DC, 512], f32r, tag="ktl")
                    nc.sync.dma_start(
                        kth[:], kthi_d.opt().rearrange(
                            "(c p) i -> p c i", p=P))
                    nc.sync.dma_start(
                        ktl[:], ktlo_d.opt().rearrange(
                            "(c p) i -> p c i", p=P))
                    for mc in range(DC):
                        csl = slice(mc * P, (mc + 1) * P)
                        pt = tp.tile([P, 512], f32, tag="pt")
                        for jc in range(DC):
                            qh_ = t2b.tile([P, P], f32r, tag="mqb_hi")
                            ql_ = t2b.tile([P, P], f32r, tag="mqb_lo")
                            nc.sync.dma_start(
                                qh_[:], mqshi_d[jc * P:(jc + 1) * P, csl])
                            nc.sync.dma_start(
                                ql_[:], mqslo_d[jc * P:(jc + 1) * P, csl])
                            nc.tensor.matmul(pt[:], qh_[:], kth[:, jc],
                                             start=(jc == 0), stop=False)
                            nc.tensor.matmul(pt[:], qh_[:], ktl[:, jc],
                                             start=False, stop=False)
                            nc.tensor.matmul(pt[:], ql_[:], kth[:, jc],
                                             start=False,
                                             stop=(jc == DC - 1))
                        hi_e = t2w.tile([P, 512], f32r, tag="t2hi_e")
                        lo_e = t2w.tile([P, 512], f32r, tag="t2lo_e")
                        _split(nc, hi_e[:], lo_e[:], pt[:])
                        nc.sync.dma_start(
                            t2hi_d[mc * P:(mc + 1) * P, :], hi_e[:])
                        nc.sync.dma_start(
                            t2lo_d[mc * P:(mc + 1) * P, :], lo_e[:])

                # ---- G fold + rowsums + P4 under one pool scope ----
                with tc.tile_pool(name="gk", bufs=1) as gk:
                    g_hi = gk.tile([P, DC, D], f32r, tag="g_hi")
                    g_lo = gk.tile([P, DC, D], f32r, tag="g_lo")
                    sx_sb = gk.tile([1, NSEQ], f32, tag="sx_sb")
                    sxr_sb = gk.tile([1, NSEQ], f32, tag="sxr_sb")
                    with tc.tile_pool(name="gs", bufs=1) as gs, \
                         tc.tile_pool(name="gb", bufs=3) as gb, \
                         tc.tile_pool(name="gw", bufs=2) as gw, \
                         tc.tile_pool(name="gxp", bufs=2) as gxp, \
                         tc.tile_pool(name="gps", bufs=4, space="PSUM") as gp, \
                         tc.tile_pool(name="gpx", bufs=1, space="PSUM") as gpx:
                        psx = [gpx.tile([1, 512], f32, tag=f"psx{q}",
                                        name=f"psx{q}") for q in range(4)]
                        t2h = gs.tile([P, DC, 512], f32r, tag="t2h")
                        t2l = gs.tile([P, DC, 512], f32r, tag="t2l")
                        nc.sync.dma_start(
                            t2h[:], t2hi_d.opt().rearrange(
                                "(c p) i -> p c i", p=P))
                        nc.sync.dma_start(
                            t2l[:], t2lo_d.opt().rearrange(
                                "(c p) i -> p c i", p=P))
                        for mc in range(DC):
                            isl = slice(mc * P, (mc + 1) * P)
                            pg = gp.tile([P, 512], f32, tag="pg")
                            for cc in range(DC):
                                wh_ = gb.tile([P, P], f32r, tag="wqb_hi")
                                wl_ = gb.tile([P, P], f32r, tag="wqb_lo")
                                nc.sync.dma_start(
                                    wh_[:],
                                    wqshi_d[cc * P:(cc + 1) * P, isl])
                                nc.sync.dma_start(
                                    wl_[:],
                                    wqslo_d[cc * P:(cc + 1) * P, isl])
                                nc.tensor.matmul(pg[:], wh_[:],
                                                 t2h[:, cc],
                                                 start=(cc == 0),
                                                 stop=False)
                                nc.tensor.matmul(pg[:], wh_[:],
                                                 t2l[:, cc],
                                                 start=False, stop=False)
                                nc.tensor.matmul(pg[:], wl_[:],
                                                 t2h[:, cc],
                                                 start=False,
                                                 stop=(cc == DC - 1))
                            g32 = gw.tile([P, 512], f32, tag="g32")
                            nc.scalar.activation(g32[:], pg[:], FP.Copy,
                                                 scale=1.0 / 32.0)
                            nc.sync.dma_start(gx_in[mc * P:(mc + 1) * P, :],
                                              g32[:])
                            # interleaved x-rowsum: one xT block per mc
                            xc = gxp.tile([P, NSEQ], f32, tag="gx_xc",
                                          bufs=1)
                            nc.sync.dma_start(xc[:],
                                              xT[mc * P:(mc + 1) * P, :])
                            xc_r = gxp.tile([P, NSEQ], f32r, tag="gx_xcr",
                                            bufs=1)
                            nc.vector.tensor_copy(xc_r[:], xc[:])
                            for q in range(4):
                                nc.tensor.matmul(
                                    psx[q][:], ones_col_r[:],
                                    xc_r[:, q * 512:(q + 1) * 512],
                                    start=(mc == 0), stop=(mc == DC - 1))
                        for q in range(4):
                            nc.scalar.copy(sx_sb[:, q * 512:(q + 1) * 512],
                                           psx[q][:])
                        nc.gpsimd.collective_compute(
                            "AllGather", mybir.AluOpType.bypass,
                            ins=[gx_in.opt()], outs=[gx_out.opt()],
                            replica_groups=PAIRS)
                        for half in range(2):
                            hsl = slice(half * 512, (half + 1) * 512)
                            gf = gs.tile([P, DC, 512], f32, tag="gf",
                                         bufs=2)
                            nc.sync.dma_start(
                                gf[:], gx_out.opt().rearrange(
                                    "(h c p) s -> p h c s",
                                    h=2, p=P)[:, half])
                            for cc in range(DC):
                                _split(nc, g_hi[:, cc, hsl],
                                       g_lo[:, cc, hsl], gf[:, cc])

                    # ---- P4: uT projection + vh projection + xr rowsum ----
                    with tc.tile_pool(name="p4", bufs=1) as p4, \
                         tc.tile_pool(name="p4w", bufs=2) as p4w, \
                         tc.tile_pool(name="p4ps", bufs=3,
                                      space="PSUM") as p4p:
                        nc.sync.dma_start(
                            vt_sb[:], vtx_out.opt().rearrange(
                                "(c p) z -> p c z", p=P).bitcast(f32r))
                        for ns in range(4):
                            nsl = slice(ns * 512, (ns + 1) * 512)
                            xh = p4.tile([P, DC, 512], f32r, tag="xh")
                            xl = p4.tile([P, DC, 512], f32r, tag="xl")
                            nc.sync.dma_start(
                                xh[:], xrhi_d.opt().rearrange(
                                    "(c p) n -> p c n", p=P)[:, :, nsl])
                            nc.sync.dma_start(
                                xl[:], xrlo_d.opt().rearrange(
                                    "(c p) n -> p c n", p=P)[:, :, nsl])
                            psxr_t = p4p.tile([1, 512], f32, tag="psxr",
                                              bufs=2)
                            for cc in range(DC):
                                nc.tensor.matmul(psxr_t[:], ones_col_r[:],
                                                 xh[:, cc],
                                                 start=(cc == 0),
                                                 stop=(cc == DC - 1))
                            nc.scalar.copy(sxr_sb[:, nsl], psxr_t[:])
                            for mc in range(DC):
                                msl = slice(mc * P, (mc + 1) * P)
                                pu = p4p.tile([P, 512], f32, tag="pu")
                                for cc in range(DC):
                                    nc.tensor.matmul(pu[:],
                                                     g_hi[:, cc, msl],
                                                     xh[:, cc],
                                                     start=(cc == 0),
                                                     stop=False)
                                    nc.tensor.matmul(pu[:],
                                                     g_hi[:, cc, msl],
                                                     xl[:, cc],
                                                     start=False, stop=False)
                                    nc.tensor.matmul(pu[:],
                                                     g_lo[:, cc, msl],
                                                     xh[:, cc],
                                                     start=False,
                                                     stop=(cc == DC - 1))
                                uhi = p4w.tile([P, 512], f32r, tag="uhi")
                                ulo = p4w.tile([P, 512], f32r, tag="ulo")
                                _split(nc, uhi[:], ulo[:], pu[:])
                                nc.sync.dma_start(
                                    uhi_d[mc * P:(mc + 1) * P, nsl], uhi[:])
                                nc.sync.dma_start(
                                    ulo_d[mc * P:(mc + 1) * P, nsl], ulo[:])
                            for nb in range(4):
                                nch = ns * 4 + nb
                                for jst in range(2):
                                    jsl = slice(jst * 512, (jst + 1) * 512)
                                    pvh = p4p.tile([P, 512], f32, tag="pvh")
                                    for cc in range(DC):
                                        nc.tensor.matmul(
                                            pvh[:],
                                            xh[:, cc, nb * P:(nb + 1) * P],
                                            vt_sb[:, cc, jsl],
                                            start=(cc == 0),
                                            stop=(cc == DC - 1))
                                    vh_e = p4w.tile([P, 512], f16,
                                                    tag="vh_e")
                                    nc.scalar.copy(vh_e[:], pvh[:])
                                    nc.sync.dma_start(
                                        vh_d[nch * P:(nch + 1) * P, jsl],
                                        vh_e[:])
                        nc.vector.tensor_mul(sx_sb[:], sx_sb[:], sxr_sb[:])
                        nc.vector.tensor_scalar_mul(sx_sb[:], sx_sb[:], 0.25)
                        nc.sync.dma_start(s_d[:], sx_sb[:])
                        nc.sync.dma_start(
                            s_sb[:],
                            s_d.opt().rearrange("o (a p) -> (o p) a", p=P))

            # ======== P5: attention + fused P7 + per-group RS =============
            # xr hi/lo streamed per key chunk and reused across the group's 4
            # query blocks. ctx @ Wof runs inside each group so the group's
            # ReduceScatter overlaps the remaining groups' compute.
            with tc.tile_pool(name="atl", bufs=2) as atl, \
                 tc.tile_pool(name="atg", bufs=1) as atg, \
                 tc.tile_pool(name="atu", bufs=1) as atu, \
                 tc.tile_pool(name="atw", bufs=1) as atw, \
                 tc.tile_pool(name="atv", bufs=2) as atv, \
                 tc.tile_pool(name="atk", bufs=1) as atk, \
                 tc.tile_pool(name="atsm", bufs=2) as atsm, \
                 tc.tile_pool(name="atps", bufs=1, space="PSUM") as atp, \
                 tc.tile_pool(name="atpc", bufs=1, space="PSUM") as atpc:
                wof_sb = atk.tile([P, DC, D], f32r, tag="wof_sb")
                nc.sync.dma_start(
                    wof_sb[:],
                    wfx_out.opt().rearrange("(c p) j -> p c j",
                                            p=P).bitcast(f32r))
                for g in range(4):
                    nkb = 4 * g + 4
                    ug_hi = atu.tile([P, DC, 512], f32r, tag="ug_hi")
                    ug_lo = atu.tile([P, DC, 512], f32r, tag="ug_lo")
                    nc.sync.dma_start(
                        ug_hi[:], uhi_d.opt().rearrange(
                            "(c p) n -> p c n",
                            p=P)[:, :, g * 512:(g + 1) * 512])
                    nc.sync.dma_start(
                        ug_lo[:], ulo_d.opt().rearrange(
                            "(c p) n -> p c n",
                            p=P)[:, :, g * 512:(g + 1) * 512])
                    sf = [atw.tile([P, (13 + iq) * P], f32, tag=f"sf{iq}",
                                   name=f"sf{iq}") for iq in range(4)]
                    for st in range(g + 1):
                        xh_s = atl.tile([P, DC, 512], f32r, tag="xh_s")
                        xl_s = atl.tile([P, DC, 512], f32r, tag="xl_s")
                        nc.sync.dma_start(
                            xh_s[:], xrhi_d.opt().rearrange(
                                "(c p) n -> p c n",
                                p=P)[:, :, st * 512:(st + 1) * 512])
                        nc.sync.dma_start(
                            xl_s[:], xrlo_d.opt().rearrange(
                                "(c p) n -> p c n",
                                p=P)[:, :, st * 512:(st + 1) * 512])
                        for iq in range(4):
                            i = 4 * g + iq
                            W = (i + 1) * P
                            wst = min(512, W - st * 512)
                            ssl = slice(st * 512, st * 512 + wst)
                            iqsl = slice(iq * P, (iq + 1) * P)
                            pS = atp.tile([P, 512], f32, tag="pS", bufs=2)
                            for cc in range(DC):
                                nc.tensor.matmul(
                                    pS[:, :wst], ug_hi[:, cc, iqsl],
                                    xh_s[:, cc, :wst],
                                    start=(cc == 0), stop=False)
                                nc.tensor.matmul(
                                    pS[:, :wst], ug_hi[:, cc, iqsl],
                                    xl_s[:, cc, :wst],
                                    start=False, stop=False)
                                nc.tensor.matmul(
                                    pS[:, :wst], ug_lo[:, cc, iqsl],
                                    xh_s[:, cc, :wst],
                                    start=False, stop=(cc == DC - 1))
                            nc.scalar.copy(sf[iq][:, ssl], pS[:, :wst])
                    at16 = atg.tile([P, NCH, 512], f16, tag="at16")
                    for iq in range(4):
                        i = 4 * g + iq
                        W = (i + 1) * P
                        nc.vector.tensor_add(sf[iq][:, i * P:(i + 1) * P],
                                             sf[iq][:, i * P:(i + 1) * P],
                                             mask_sb[:])
                        m_ = atsm.tile([P, 1], f32, tag="rowmax")
                        nc.vector.reduce_max(m_[:], sf[iq][:, 0:W], axis=X)
                        negm = atsm.tile([P, 1], f32, tag="negm")
                        nc.vector.tensor_scalar_mul(negm[:], m_[:], -1.0)
                        a_r = atsm.tile([P, NSEQ], f16, tag="a_r", bufs=1)
                        rsum = atsm.tile([P, 1], f32, tag="rsum")
                        nc.scalar.activation(a_r[:, 0:W], sf[iq][:, 0:W],
                                             FP.Exp, bias=negm[:],
                                             accum_out=rsum[:])
                        rinv = atsm.tile([P, 1], f32, tag="rinv")
                        nc.vector.reciprocal(rinv[:], rsum[:])
                        nc.vector.tensor_mul(coef_sb[:, i:i + 1], rinv[:],
                                             s_sb[:, i:i + 1])
                        for kb in range(i + 1):
                            pT = atp.tile([P, P], f16, tag="pT", bufs=1)
                            nc.tensor.transpose(
                                pT[:], a_r[:, kb * P:(kb + 1) * P],
                                ident16[:])
                            nc.vector.tensor_copy(
                                at16[:, kb, iq * P:(iq + 1) * P], pT[:])
                        for kb in range(i + 1, nkb):
                            nc.vector.memset(
                                at16[:, kb, iq * P:(iq + 1) * P], 0.0)
                    ctx_sb = atk.tile([P, DC, 512], f32r, tag="ctx_sb")
                    for zzh in range(2):
                        zsl = slice(zzh * 512, (zzh + 1) * 512)
                        pCs = [atpc.tile([P, 512], f32, tag=f"pC{z4}",
                                         name=f"pC{z4}") for z4 in range(4)]
                        for kb in range(nkb):
                            vt16 = atv.tile([P, 512], f16, tag="vt16")
                            nc.sync.dma_start(vt16[:],
                                              vh_d[kb * P:(kb + 1) * P, zsl])
                            for z4 in range(4):
                                nc.tensor.matmul(
                                    pCs[z4][:],
                                    vt16[:, z4 * P:(z4 + 1) * P],
                                    at16[:, kb, :],
                                    start=(kb == 0), stop=(kb == nkb - 1))
                        for z4 in range(4):
                            nc.vector.tensor_copy(ctx_sb[:, zzh * 4 + z4],
                                                  pCs[z4][:])
                    # fused P7 for this query group + ReduceScatter
                    for iq in range(4):
                        i = 4 * g + iq
                        ob = atsm.tile([P, D], f32, tag="ob", bufs=1)
                        for jst in range(2):
                            jsl = slice(jst * 512, (jst + 1) * 512)
                            po = atp.tile([P, 512], f32, tag="po", bufs=1)
                            for zz in range(DC):
                                nc.tensor.matmul(
                                    po[:],
                                    ctx_sb[:, zz, iq * P:(iq + 1) * P],
                                    wof_sb[:, zz, jsl],
                                    start=(zz == 0), stop=(zz == DC - 1))
                            nc.scalar.activation(ob[:, jsl], po[:], FP.Copy,
                                                 scale=coef_sb[:, i:i + 1])
                        nc.sync.dma_start(
                            outg_d[g][iq * P:(iq + 1) * P, :], ob[:])
                    nc.gpsimd.collective_compute(
                        "ReduceScatter", mybir.AluOpType.add,
                        ins=[outg_d[g].opt()], outs=[rsg_d[g].opt()],
                        replica_groups=GROUPS)
                for g in range(4):
                    nc.sync.dma_start(out_shards[g][:], rsg_d[g][:])

    nc.compile()
    return nc


def _host_tables():
    half = D // 2
    pos = np.arange(NSEQ, dtype=np.float32)
    inv = (np.float32(1.0)
           / (np.float32(10000.0)
              ** (np.arange(half, dtype=np.float32) / np.float32(half))))
    inv = inv.astype(np.float32)
    ang = (pos[:, None] * inv[None, :]).astype(np.float32)
    cosT = np.ascontiguousarray(np.cos(ang).astype(np.float32).T)
    sinT = np.ascontiguousarray(np.sin(ang).astype(np.float32).T)
    r = np.arange(P)
    mask = np.where(r[:, None] >= r[None, :], np.float32(0.0),
                    np.float32(-1e30)).astype(np.float32)
    return cosT, sinT, mask


def build_in_maps(inputs):
    x = np.asarray(inputs["x"], dtype=np.float32)
    cosT, sinT, mask = _host_tables()
    c = np.ascontiguousarray

    wqTn = c(np.asarray(inputs["wq"], np.float32).T)
    wkTn = c(np.asarray(inputs["wk"], np.float32).T)
    wvTn = c(np.asarray(inputs["wv"], np.float32).T)
    # fan-in halves swapped for cores 4-7: they compute the second half of
    # the pair-split folds while running the same SPMD program.
    wkTs = c(np.concatenate([wkTn[:, HB:], wkTn[:, :HB]], axis=1))
    wvTs = c(np.concatenate([wvTn[:, HB:], wvTn[:, :HB]], axis=1))
    mwq_n = np.asarray(inputs["mwq"], np.float32)
    mwk_n = np.asarray(inputs["mwk"], np.float32)
    mwv_n = np.asarray(inputs["mwv"], np.float32)
    mwo_n = np.asarray(inputs["mwo"], np.float32)
    wf_n = c(np.asarray(inputs["wf"], np.float32))

    in_maps = []
    for core in range(N_CORES):
        b, t = core // 4, core % 4
        zsl = slice(t * ZT, (t + 1) * ZT)
        zhs = slice(t * ZT + b * HB, t * ZT + (b + 1) * HB)
        in_maps.append({
            "xT": c(x[b].T),
            "cosT": cosT,
            "sinT": sinT,
            "wqT": wqTn,
            "wkT": wkTn if b == 0 else wkTs,
            "wvT": wvTn if b == 0 else wvTs,
            "mwqT": c(mwq_n[:, zsl].T),
            "mwk": c(mwk_n[:, zsl]),
            "mwv": c(mwv_n[:, zsl]),
            "mwoT": c(mwo_n[zhs, :].T),
            "wf": wf_n,
            "maskc": mask,
        })
    return in_maps


def kernel(**inputs):
    if "nc" not in _CACHE:
        _CACHE["nc"] = build_nc()
    nc = _CACHE["nc"]
    in_maps = build_in_maps(inputs)

    res = run_bass_kernel_spmd(nc, in_maps, list(range(N_CORES)))

    out = np.empty((2, NSEQ, D), dtype=np.float32)
    for core in range(N_CORES):
        b, t = core // 4, core % 4
        for g in range(4):
            r0 = g * 512 + t * P
            out[b, r0:r0 + P, :] = res.results[core][f"out_shard{g}"]
    return out

